# revision 13
# baseline (speedup 1.0000x reference)
"""Trainium2 Bass kernel for nn_Decoder (512-step LSTM scan, B=256, F=256).

Strategy: data-parallel over batch across 8 NeuronCores (32 batch/core).
After step 0 the LSTM input equals the hidden state, so W_ih+W_hh fold into
one combined weight for steps 1..511. Step 0 (and the initial_layer) runs on
host in numpy; each core runs `steps` uniform recurrence steps.

v2: feature-major dataflow. Gates are computed as [gate-rows, batch] tiles
(weights stationary, h streaming), so
  - every elementwise op is a [128, 64] tile (free dim 64, full partitions)
  - h feeds the next matmul directly -- no per-step transpose on the h path
  - bias is injected by one-hot matmuls (off the critical path)
  - gate order g,i,f,o so the c-path's inputs are computed first
  - h accumulates in rotating 4-step "quad" tiles; each completed quad is
    PE-transposed (2 matmuls) to batch-major and ACT-copied into an SBUF
    mega-buffer; one DMA ships all outputs at the end.

Per-step critical chain: PE gate MMs -> ACT tanh(g)/sig(i) -> DVE t1 ->
DVE c2 -> ACT tanh(c2) -> DVE h2 -> PE (next step).
"""
import sys

sys.path.insert(0, "/opt/trn_rl_repo")

import numpy as np

SEQ_LEN = 512
B, L, F = 256, 128, 256
NCORES = 8
BS = B // NCORES  # 32 batch per core

_CACHE = {}
VERSION = 9  # bump on every program change: forces a distinct NEFF cache key

# gate-chunk order in wsb/b8: i0 i1 f0 f1 g0 g1 o0 o1 (natural Wc row order)


def _sigmoid(x):
    out = np.empty_like(x)
    pos = x >= 0
    out[pos] = 1.0 / (1.0 + np.exp(-x[pos]))
    e = np.exp(x[~pos])
    out[~pos] = e / (1.0 + e)
    return out


def _build(steps):
    """Build + schedule the per-core Bass program (same program all cores)."""
    import concourse.mybir as mybir
    import concourse.tile as tile
    from concourse import bacc

    f32 = mybir.dt.float32
    f32r = mybir.dt.float32r
    AF = mybir.ActivationFunctionType

    nc = bacc.Bacc("TRN2", target_bir_lowering=False, debug=False)

    hT0_d = nc.dram_tensor("hT0", [128, 2 * BS], f32, kind="ExternalInput")
    c0_d = nc.dram_tensor("c0f", [128, 2 * BS], f32, kind="ExternalInput")
    wsb_d = nc.dram_tensor("wsb", [128, 2048], f32, kind="ExternalInput")
    b8_d = nc.dram_tensor("b8", [8, 128], f32, kind="ExternalInput")
    eh_d = nc.dram_tensor("eh", [8, 8 * BS], f32, kind="ExternalInput")
    id_d = nc.dram_tensor("ident", [128, 128], f32, kind="ExternalInput")
    # cache-buster: the neuron NEFF cache key ignores backend_config (the BIR),
    # so distinct programs with identical I/O shapes collide. Unique shape per
    # (VERSION, steps) forces a distinct HLO and cache entry.
    stag_d = nc.dram_tensor("stag", [VERSION, steps], f32, kind="ExternalInput")
    outs_d = nc.dram_tensor("outs", [SEQ_LEN, BS, F], f32, kind="ExternalOutput")

    nquad = (steps + 3) // 4
    n_mega = nquad * F  # fp32 cols per partition in mega buffer

    with tile.TileContext(nc) as tc:
        with tc.tile_pool(name="const", bufs=1) as cpool, \
             tc.tile_pool(name="hq", bufs=3) as hqpool, \
             tc.tile_pool(name="cst", bufs=2) as cspool, \
             tc.tile_pool(name="work", bufs=2) as wpool, \
             tc.tile_pool(name="psg", bufs=2, space="PSUM") as psp, \
             tc.tile_pool(name="pst", bufs=2, space="PSUM") as ptp:

            wsb = cpool.tile([128, 2048], f32r)
            nc.gpsimd.dma_start(out=wsb[:], in_=wsb_d.ap())
            b8 = cpool.tile([8, 128], f32r)
            nc.gpsimd.dma_start(out=b8[:], in_=b8_d.ap())
            eh = cpool.tile([8, 256], f32r)
            nc.gpsimd.dma_start(out=eh[:], in_=eh_d.ap())
            ident = cpool.tile([128, 128], f32r)
            nc.gpsimd.dma_start(out=ident[:], in_=id_d.ap())
            mega = cpool.tile([128, n_mega], f32)
            stag_sb = cpool.tile([1, 1], f32)
            nc.sync.dma_start(out=stag_sb[:], in_=stag_d.ap()[0:1, 0:1])

            h0 = cpool.tile([128, 64], f32r)
            nc.gpsimd.dma_start(out=h0[:], in_=hT0_d.ap())
            c_cur = cspool.tile([128, 64], f32, tag="c")
            nc.sync.dma_start(out=c_cur[:], in_=c0_d.ap())

            quads = []   # [(hq_tile, n_slots)] pending transpose
            hq_cur = None

            def emit_quad_transpose():
                """Transpose the oldest completed quad into b-major PSUM and
                return the ACT mega-copy closure."""
                hq, dq, nslots = quads.pop(0)
                tps = ptp.tile([128, 256], f32r, tag="tps")
                for k in range(2):
                    # hq col layout 128*kk + 32*s + b: chunk k's slots are a
                    # contiguous [128, 32*nslots] slab -> 1-D stationary AP
                    nc.tensor.matmul(
                        tps[0:32 * nslots, 128 * k:128 * k + 128],
                        lhsT=hq[:, 128 * k:128 * k + 32 * nslots],
                        rhs=ident[:], is_transpose=True)
                def do_copy():
                    nc.vector.tensor_scalar_add(
                        mega[0:32 * nslots, 256 * dq:256 * dq + 256],
                        tps[0:32 * nslots, :], 0.0)
                return do_copy, dq

            shipped = [0]  # quads already sent to DRAM

            def ship_quads(q_hi):
                """DMA mega cols for full quads [shipped, q_hi) to outs."""
                q_lo = shipped[0]
                if q_hi <= q_lo:
                    return
                nc.sync.dma_start(
                    out=outs_d.ap()[4 * q_lo + 1:4 * q_hi + 1].rearrange(
                        "(d g) b f -> (g b) d f", g=4),
                    in_=mega[:, 256 * q_lo:256 * q_hi].rearrange(
                        "p (d f) -> p d f", f=256))
                shipped[0] = q_hi

            pending_copy = None
            for t in range(1, steps + 1):
                q = (t - 1) % 4
                if q == 0:
                    hq_prev_tile = hq_cur
                    hq_cur = hqpool.tile([128, 256], f32r, tag="hq")
                # h(t-1) source per k-chunk (hq col layout: 128*kk + 32*s + b)
                if t == 1:
                    h_in_k = [h0[:, 0:32], h0[:, 32:64]]
                elif (t - 2) % 4 == 3:
                    h_in_k = [hq_prev_tile[:, 96:128],
                              hq_prev_tile[:, 224:256]]
                else:
                    sp_ = 32 * ((t - 2) % 4)
                    h_in_k = [hq_cur[:, sp_:sp_ + 32],
                              hq_cur[:, 128 + sp_:128 + sp_ + 32]]

                psIF = psp.tile([128, 128], f32, tag="psIF")
                psG = psp.tile([128, 64], f32, tag="psG")
                psO = psp.tile([128, 64], f32, tag="psO")

                # bias via one-hot matmuls (no dependency on h -> runs early)
                nc.tensor.matmul(psIF[:], lhsT=b8[:], rhs=eh[:, 0:128],
                                 start=True, stop=False)
                nc.tensor.matmul(psG[:], lhsT=b8[:], rhs=eh[:, 128:192],
                                 start=True, stop=False)
                nc.tensor.matmul(psO[:], lhsT=b8[:], rhs=eh[:, 192:256],
                                 start=True, stop=False)

                def gate_mms(ps, j0, col0=0):
                    for jj in range(2):
                        j = j0 + jj
                        for k in range(2):
                            nc.tensor.matmul(
                                ps[:, col0 + 32 * jj:col0 + 32 * jj + 32],
                                lhsT=wsb[:, 256 * j + 128 * k:
                                         256 * j + 128 * k + 128],
                                rhs=h_in_k[k],
                                start=False, stop=(k == 1),
                                skip_group_check=True)

                gate_mms(psIF, 0)          # i -> cols 0:64
                gate_mms(psIF, 2, col0=64)  # f -> cols 64:128
                gate_mms(psG, 4)
                # completed quad -> b-major transpose (PE idle slot here;
                # delays only the o-gate MMs, which have slack)
                if quads:
                    pending_copy = emit_quad_transpose()
                gate_mms(psO, 6)

                sif = wpool.tile([128, 128], f32, tag="sif")
                nc.scalar.activation(sif[:], psIF[:], AF.Sigmoid)
                tg = wpool.tile([128, 64], f32, tag="tg")
                nc.scalar.activation(tg[:], psG[:], AF.Tanh)

                t2 = wpool.tile([128, 64], f32, tag="t2")
                nc.vector.tensor_mul(t2[:], sif[:, 64:128], c_cur[:])
                t1 = wpool.tile([128, 64], f32, tag="t1")
                nc.vector.tensor_mul(t1[:], sif[:, 0:64], tg[:])
                c_new = cspool.tile([128, 64], f32, tag="c")
                nc.vector.tensor_add(c_new[:], t1[:], t2[:])

                so = wpool.tile([128, 64], f32, tag="so")
                nc.scalar.activation(so[:], psO[:], AF.Sigmoid)
                tcx = wpool.tile([128, 64], f32, tag="tcx")
                nc.scalar.activation(tcx[:], c_new[:], AF.Tanh)

                # h2 -> quad slot q: cols (32q:32q+32, 128+32q:128+32q+32)
                h_new = hq_cur[:].rearrange(
                    "p (kk sb) -> p kk sb", kk=2)[:, :, 32 * q:32 * q + 32]
                so2 = so[:].rearrange("p (kk b) -> p kk b", kk=2)
                tcx2 = tcx[:].rearrange("p (kk b) -> p kk b", kk=2)
                nc.vector.tensor_mul(h_new, so2, tcx2)

                if q == 3:
                    quads.append((hq_cur, (t - 1) // 4, 4))

                if pending_copy is not None:
                    pending_copy[0]()
                    if (pending_copy[1] + 1) % 32 == 0:
                        ship_quads(pending_copy[1] + 1)
                    pending_copy = None

                c_cur = c_new

            if steps % 4:
                quads.append((hq_cur, (steps - 1) // 4, steps % 4))
            while quads:
                pending_copy = emit_quad_transpose()
                pending_copy[0]()

            # ship the remaining mega cols: full quads, then the partial tail
            full_d = steps // 4
            rem = steps % 4
            ship_quads(full_d)
            if rem:
                nc.sync.dma_start(
                    out=outs_d.ap()[4 * full_d + 1: steps + 1].rearrange(
                        "g b f -> (g b) f"),
                    in_=mega[0:32 * rem, 256 * full_d:256 * full_d + 256])

    nc.compile()
    return nc


def _get_nc(steps):
    if steps not in _CACHE:
        _CACHE[steps] = _build(steps)
    return _CACHE[steps]


def _host_prep(x, last_feat, Wi, bi, W_ih, W_hh, b_ih, b_hh):
    x = np.asarray(x, np.float32)
    last_feat = np.asarray(last_feat, np.float32)
    Wi = np.asarray(Wi, np.float32); bi = np.asarray(bi, np.float32)
    W_ih = np.asarray(W_ih, np.float32); W_hh = np.asarray(W_hh, np.float32)
    b_ih = np.asarray(b_ih, np.float32); b_hh = np.asarray(b_hh, np.float32)

    z = x[0] @ Wi.T + bi                       # [B, F]
    init = np.where(z > 0, z, np.expm1(z)).astype(np.float32)  # elu

    bsum = b_ih + b_hh
    g0 = last_feat @ W_ih.T + init @ W_hh.T + bsum   # [B, 4F] order i,f,g,o
    i0, f0, g0g, o0 = (g0[:, 0:F], g0[:, F:2*F], g0[:, 2*F:3*F], g0[:, 3*F:4*F])
    c1 = _sigmoid(f0) * init + _sigmoid(i0) * np.tanh(g0g)
    h1 = (_sigmoid(o0) * np.tanh(c1)).astype(np.float32)
    c1 = c1.astype(np.float32)

    Wc = W_ih + W_hh                            # [4F, F] gate order i,f,g,o
    WcT = np.ascontiguousarray(Wc.T)            # [F, 4F]
    # stationary chunks: wsb[p, 256j+128k+c] = WcT[128k+p, 128j+c]
    w4 = WcT.reshape(2, 128, 8, 128)                          # [k, p, j, c]
    wsb = np.ascontiguousarray(w4.transpose(1, 2, 0, 3).reshape(128, 2048))
    b8 = np.ascontiguousarray(bsum.reshape(8, 128))
    return h1, c1, wsb, b8


def _fmajor(a):
    """[BS, F] batch-major -> [128, 2*BS] feature-major packed chunks."""
    # out[p, 32k+b] = a[b, 128k+p]
    return np.ascontiguousarray(
        a.T.reshape(2, 128, BS).transpose(1, 0, 2).reshape(128, 2 * BS))


def kernel(x, last_feat, Wi, bi, W_ih, W_hh, b_ih, b_hh, Wo, bo,
           _steps=SEQ_LEN - 1):
    from concourse.bass_utils import run_bass_kernel_spmd

    h1, c1, wsb, b8 = _host_prep(x, last_feat, Wi, bi, W_ih, W_hh,
                                 b_ih, b_hh)
    eh = np.repeat(np.eye(8, dtype=np.float32), BS, axis=1)  # [8, 256]
    ident = np.eye(128, dtype=np.float32)
    stag = np.zeros((VERSION, _steps), np.float32)
    in_maps = []
    for ci in range(NCORES):
        s = slice(ci * BS, (ci + 1) * BS)
        in_maps.append(dict(
            hT0=_fmajor(h1[s]), c0f=_fmajor(c1[s]),
            wsb=wsb, b8=b8, eh=eh, ident=ident, stag=stag))

    nc = _get_nc(_steps)
    res = run_bass_kernel_spmd(nc, in_maps, core_ids=list(range(NCORES)))

    outs = np.concatenate([r["outs"] for r in res.results], axis=1)  # [S, B, F]
    outs[0] = h1
    return np.ascontiguousarray(outs).reshape(B, SEQ_LEN, F)
